# revision 36
# baseline (speedup 1.0000x reference)
import sys

sys.path.insert(0, "/opt/trn_rl_repo")
import numpy as np
import ml_dtypes
import concourse.mybir as mybir
from concourse import bacc
from concourse.tile import TileContext
from concourse.bass_utils import run_bass_kernel_spmd

F32 = mybir.dt.float32
BF16 = mybir.dt.bfloat16
EXP = mybir.ActivationFunctionType.Exp

B, S, D = 4, 2048, 1024
NH, HD = 16, 64


def build(reps=1):
    nc = bacc.Bacc()
    qx = nc.declare_dram_parameter("qx", [128, 8, 2048], BF16, isOutput=False)
    kx = nc.declare_dram_parameter("kx", [128, 8, 2048], BF16, isOutput=False)
    vx = nc.declare_dram_parameter("vx", [128, 8, 2048], BF16, isOutput=False)
    wq = nc.declare_dram_parameter("wq", [128, 8, 512], BF16, isOutput=False)
    wk = nc.declare_dram_parameter("wk", [128, 8, 512], BF16, isOutput=False)
    wv = nc.declare_dram_parameter("wv", [128, 8, 512], BF16, isOutput=False)
    wo = nc.declare_dram_parameter("wo", [128, 8, 512], BF16, isOutput=False)
    yT = nc.declare_dram_parameter("yT", [128, 8, 2048], BF16, isOutput=True)

    with TileContext(nc) as tc:
        with tc.sbuf_pool(name="sb", bufs=1) as pool, tc.psum_pool(
            name="ps", bufs=1
        ) as pp:
            for _rep in range(reps):
                wq_sb = pool.tile([128, 8, 512], BF16, tag="wq")
                wk_sb = pool.tile([128, 8, 512], BF16, tag="wk")
                wv_sb = pool.tile([128, 8, 512], BF16, tag="wv")
                wo_sb = pool.tile([128, 8, 512], BF16, tag="wo")
                nc.sync.dma_start(out=wk_sb[:], in_=wk[:])

                qt = [
                    pool.tile([128, 2048], BF16, tag=f"qt{r}", name=f"qt{r}")
                    for r in range(4)
                ]
                kt = [
                    pool.tile([128, 2048], BF16, tag=f"kt{r}", name=f"kt{r}")
                    for r in range(4)
                ]
                v_sb = pool.tile([128, 16, 8, 65], BF16, tag="vsb")
                nc.vector.memset(v_sb[:], 1.0)
                ones = pool.tile([1, 64], BF16, tag="ones")
                nc.vector.memset(ones[:], 1.0)

                # streamed input chunks; tags sized to peak liveness
                it_map = {}

                def get_in(xkey, c):
                    if (xkey, c) not in it_map:
                        xin, tag, nb = {
                            "q": (qx, "inq", 4),
                            "k": (kx, "ink", 4),
                            "v": (vx, "inv", 2),
                        }[xkey]
                        i_t = pool.tile(
                            [128, 8, 512], BF16, tag=tag, bufs=nb,
                            name=f"i{xkey}{c}",
                        )
                        nc.sync.dma_start(
                            out=i_t[:], in_=xin[:, :, 512 * c : 512 * (c + 1)]
                        )
                        it_map[(xkey, c)] = i_t
                    return it_map[(xkey, c)]

                def proj_sub(xkey, win, dst, c, r):
                    # project input seq-chunk c for head pair r
                    i_t = get_in(xkey, c)
                    p = pp.tile([128, 512], F32, tag="pp", bufs=2)
                    for kc in range(8):
                        nc.tensor.matmul(
                            p[:],
                            win[:, kc, 128 * r : 128 * (r + 1)],
                            i_t[:, kc, :],
                            start=(kc == 0),
                            stop=(kc == 7),
                        )
                    nc.vector.tensor_copy(
                        out=dst[r][:, 512 * c : 512 * (c + 1)], in_=p[:]
                    )

                def ksub(c, r):
                    proj_sub("k", wk_sb, kt, c, r)

                def qsub(c, r):
                    proj_sub("q", wq_sb, qt, c, r)

                def vsub(c, ktl):
                    # V projection for kv chunk kti=4c+ktl, all 8 heads
                    i_t = get_in("v", c)
                    kti = 4 * c + ktl
                    p = pp.tile([128, 8, 64], F32, tag="pp", bufs=2)
                    for kc in range(8):
                        nc.tensor.matmul(
                            p[:, :, :],
                            i_t[:, kc, 128 * ktl : 128 * (ktl + 1)],
                            wv_sb[:, kc, :],
                            start=(kc == 0),
                            stop=(kc == 7),
                        )
                    nc.vector.tensor_copy(out=v_sb[:, kti, :, 0:64], in_=p[:])

                # minimal prefix: just what (qb0, r0, kti=0) needs — DMA
                # order interleaves weights with the input chunk each unit
                # actually consumes, so ksub unblocks as early as possible
                get_in("k", 0)
                nc.sync.dma_start(out=wq_sb[:], in_=wq[:])
                get_in("q", 0)
                nc.sync.dma_start(out=wv_sb[:], in_=wv[:])
                get_in("v", 0)
                nc.sync.dma_start(out=wo_sb[:], in_=wo[:])
                ksub(0, 0)
                qsub(0, 0)
                vsub(0, 0)
                get_in("k", 1)  # pre-warm next K chunk DMA

                # Group order is r-outer ("quartets"), qb-inner: quartet 0
                # carries the K/V streaming; Q chunks and later quartets' K
                # projections spread thin across the ACT-bound groups.
                # Inserts are emitted after attnV(kti) and must precede their
                # first reader: scores(r,qb,4c) needs kt[r] chunk c -> emit at
                # kti<=4c-1; attnV(kti) needs v chunk kti -> kti-1.
                inserts = {}

                def add_ins(r, qb, kti, fn):
                    inserts.setdefault((r, qb, kti), []).append(fn)

                # v chunks stream through group (0,0), one kti ahead; DMA
                # pre-warm 4 ahead
                for kti in range(15):
                    c, ktl = divmod(kti + 1, 4)
                    add_ins(0, 0, kti, lambda c=c, k=ktl: vsub(c, k))
                for c in (1, 2, 3):
                    add_ins(0, 0, 4 * (c - 1), lambda c=c: get_in("v", c))
                # kt chunks 1..3 for r0 ahead of the sweep (DMA pre-warmed)
                for c in (1, 2, 3):
                    add_ins(0, 0, 4 * c - 2, lambda c=c: ksub(c, 0))
                for c in (2, 3):
                    add_ins(0, 0, 4 * c - 6, lambda c=c: get_in("k", c))
                # qt chunk c for r=0 lands during group (0, c-1)
                for c in (1, 2, 3):
                    add_ins(0, c - 1, 4, lambda c=c: get_in("q", c))
                    add_ins(0, c - 1, 12, lambda c=c: qsub(c, 0))
                # quartets r>=1: kt/qt chunk 0 just before, rest spread
                for r in (1, 2, 3):
                    add_ins(r - 1, 3, 13, lambda r=r: ksub(0, r))
                    add_ins(r - 1, 3, 15, lambda r=r: qsub(0, r))
                    for c in (1, 2, 3):
                        add_ins(r, 0, 4 * c - 2, lambda c=c, r=r: ksub(c, r))
                        add_ins(r, c - 1, 8, lambda c=c, r=r: qsub(c, r))

                def outproj_dmc(prev_ot, qb, dmc):
                    p = pp.tile([128, 512], F32, tag="pp", bufs=2)
                    for r in range(4):
                        nc.tensor.matmul(
                            p[:],
                            wo_sb[
                                :,
                                2 * r + dmc // 4,
                                (dmc % 4) * 128 : (dmc % 4) * 128 + 128,
                            ],
                            prev_ot[r][:],
                            start=(r == 0),
                            stop=(r == 3),
                        )
                    yb = pool.tile([128, 512], BF16, tag="yb", bufs=2)
                    nc.vector.tensor_copy(out=yb[:], in_=p[:])
                    nc.sync.dma_start(
                        out=yT[:, dmc, 512 * qb : 512 * (qb + 1)], in_=yb[:]
                    )

                def norm_finish(acc_s, rec, ot_t):
                    # PE broadcast of 1/denom via ones outer product, then
                    # scale acc into ot.  Deferred into the next group's
                    # stream so the PE never waits on the DVE recip chain.
                    bc_ps = pp.tile([128, 512], F32, tag="pp", bufs=2)
                    nc.tensor.matmul(
                        bc_ps[0:64, :], ones[:], rec[0:1, 0:512],
                        start=True, stop=True,
                    )
                    nc.tensor.matmul(
                        bc_ps[64:128, :], ones[:], rec[0:1, 512:1024],
                        start=True, stop=True,
                    )
                    bcA = pool.tile([128, 512], F32, tag="bca", bufs=2)
                    nc.vector.tensor_copy(out=bcA[0:64, :], in_=bc_ps[0:64, :])
                    bcB = pool.tile([128, 512], F32, tag="bcb", bufs=2)
                    nc.vector.tensor_copy(
                        out=bcB[0:64, :], in_=bc_ps[64:128, :]
                    )
                    nc.vector.tensor_mul(
                        out=ot_t[0:64, :],
                        in0=acc_s[0:64, 0:512],
                        in1=bcA[0:64, :],
                    )
                    nc.vector.tensor_mul(
                        out=ot_t[64:128, :],
                        in0=acc_s[0:64, 512:1024],
                        in1=bcB[0:64, :],
                    )

                ot_store = {qb: [None] * 4 for qb in range(4)}
                pending_norm = None
                for r in range(4):
                    for qb in range(4):
                        acc = pp.tile([128, 1024], F32, tag="acc", bufs=1)
                        for kti in range(16):
                            s_t = pp.tile([128, 1024], F32, tag="sc", bufs=2)
                            nc.tensor.matmul(
                                s_t[:, 0:512],
                                kt[r][0:64, 128 * kti : 128 * (kti + 1)],
                                qt[r][0:64, 512 * qb : 512 * (qb + 1)],
                                start=True,
                                stop=True,
                            )
                            nc.tensor.matmul(
                                s_t[:, 512:1024],
                                kt[r][64:128, 128 * kti : 128 * (kti + 1)],
                                qt[r][64:128, 512 * qb : 512 * (qb + 1)],
                                start=True,
                                stop=True,
                            )
                            pt_t = pool.tile(
                                [128, 1024], BF16, tag="pt", bufs=3
                            )
                            nc.scalar.activation(
                                out=pt_t[:], in_=s_t[:], func=EXP, scale=0.125
                            )
                            for h in range(2):
                                nc.tensor.matmul(
                                    acc[0:65, 512 * h : 512 * (h + 1)],
                                    v_sb[:, kti, 2 * r + h, :],
                                    pt_t[:, 512 * h : 512 * (h + 1)],
                                    start=(kti == 0),
                                    stop=(kti == 15),
                                )
                            for fn in inserts.get((r, qb, kti), ()):
                                fn()
                            if pending_norm is not None and kti == 2:
                                pending_norm()
                                pending_norm = None
                            # finished q-chunks' output projections, spread
                            # through quartet 3's groups
                            if r == 3 and qb >= 1 and kti >= 5 and kti % 2 == 1:
                                outproj_dmc(
                                    ot_store[qb - 1], qb - 1, (kti - 5) // 2
                                )
                            if r == 3 and qb >= 2 and kti in (1, 3):
                                outproj_dmc(
                                    ot_store[qb - 2], qb - 2, 6 + (kti - 1) // 2
                                )
                        # fast eviction frees acc; normalize finish deferred
                        # into the next group's stream
                        rec = pool.tile([1, 1024], BF16, tag="rec", bufs=2)
                        with nc.allow_low_precision(
                            reason="softmax denom recip bf16"
                        ):
                            nc.vector.reciprocal(
                                out=rec[:], in_=acc[64:65, :]
                            )
                        acc_s = pool.tile(
                            [128, 1024], F32, tag="accs", bufs=2
                        )
                        nc.vector.tensor_copy(
                            out=acc_s[0:64, :], in_=acc[0:64, :]
                        )
                        ot_t = pool.tile([128, 512], BF16, tag="ot", bufs=16)
                        pending_norm = (
                            lambda a=acc_s, rc=rec, o=ot_t: norm_finish(
                                a, rc, o
                            )
                        )
                        ot_store[qb][r] = ot_t
                # tail: last normalize, then remaining output projections
                pending_norm()
                pending_norm = None
                for dmc in (6, 7):
                    outproj_dmc(ot_store[2], 2, dmc)
                for dmc in range(8):
                    outproj_dmc(ot_store[3], 3, dmc)
                it_map.clear()
    return nc


def _pack_in(x):  # [2048, 1024] -> [128, 8, 2048]
    return np.ascontiguousarray(x.T.reshape(8, 128, 2048).transpose(1, 0, 2))


def _pack_w(wt, g):  # W.T [1024,1024] cols for group g -> [128, 8, 512]
    return np.ascontiguousarray(
        wt[:, 512 * g : 512 * (g + 1)].reshape(8, 128, 512).transpose(1, 0, 2)
    )


def _pack_wo(wot, g):  # Wo.T rows for group g -> [128, 8, 512]
    a = wot[512 * g : 512 * (g + 1), :].reshape(4, 128, 1024).transpose(1, 0, 2)
    w8 = np.empty((128, 8, 512), np.float32)
    for r in range(4):
        for j in range(2):
            w8[:, 2 * r + j, :] = a[:, r, j * 512 : (j + 1) * 512]
    return w8


def _prepare(inputs, reps=1):
    query = np.asarray(inputs["query"], np.float32)
    key = np.asarray(inputs["key"], np.float32)
    value = np.asarray(inputs["value"], np.float32)
    WqT = np.asarray(inputs["Wq"], np.float32).T
    WkT = np.asarray(inputs["Wk"], np.float32).T
    WvT = np.asarray(inputs["Wv"], np.float32).T
    WoT = np.asarray(inputs["Wo"], np.float32).T

    bf = lambda a: a.astype(ml_dtypes.bfloat16)
    in_maps = []
    for c in range(8):
        b, g = c // 2, c % 2
        in_maps.append(
            {
                "qx": bf(_pack_in(query[b])),
                "kx": bf(_pack_in(key[b])),
                "vx": bf(_pack_in(value[b])),
                "wq": bf(_pack_w(WqT, g)),
                "wk": bf(_pack_w(WkT, g)),
                "wv": bf(_pack_w(WvT, g)),
                "wo": bf(_pack_wo(WoT, g)),
            }
        )

    nc = build(reps)
    nc.finalize()
    return nc, in_maps


def kernel(**inputs):
    nc, in_maps = _prepare(inputs)
    res = run_bass_kernel_spmd(nc, in_maps, core_ids=list(range(8)))

    out = np.empty((B, S, D), np.float32)
    for b in range(B):
        t = res.results[2 * b]["yT"].astype(np.float32) + res.results[
            2 * b + 1
        ]["yT"].astype(np.float32)
        out[b] = t.transpose(1, 0, 2).reshape(1024, 2048).T
    return out


# revision 38
# speedup vs baseline: 1.4098x; 1.4098x over previous
import sys

sys.path.insert(0, "/opt/trn_rl_repo")
import numpy as np
import ml_dtypes
import concourse.mybir as mybir
from concourse import bacc
from concourse.tile import TileContext
from concourse.bass_utils import run_bass_kernel_spmd

F32 = mybir.dt.float32
BF16 = mybir.dt.bfloat16
EXP = mybir.ActivationFunctionType.Exp

B, S, D = 4, 2048, 1024
NH, HD = 16, 64


def build(reps=1):
    nc = bacc.Bacc()
    qx = nc.declare_dram_parameter("qx", [128, 8, 2048], BF16, isOutput=False)
    kx = nc.declare_dram_parameter("kx", [128, 8, 2048], BF16, isOutput=False)
    vx = nc.declare_dram_parameter("vx", [128, 8, 2048], BF16, isOutput=False)
    wq = nc.declare_dram_parameter("wq", [128, 8, 512], BF16, isOutput=False)
    wk = nc.declare_dram_parameter("wk", [128, 8, 512], BF16, isOutput=False)
    wv = nc.declare_dram_parameter("wv", [128, 8, 512], BF16, isOutput=False)
    wo = nc.declare_dram_parameter("wo", [128, 8, 512], BF16, isOutput=False)
    yT = nc.declare_dram_parameter("yT", [128, 8, 2048], BF16, isOutput=True)

    with TileContext(nc) as tc:
        with tc.sbuf_pool(name="sb", bufs=1) as pool, tc.psum_pool(
            name="ps", bufs=1
        ) as pp:
            for _rep in range(reps):
                wq_sb = pool.tile([128, 8, 512], BF16, tag="wq")
                wk_sb = pool.tile([128, 8, 512], BF16, tag="wk")
                wv_sb = pool.tile([128, 8, 512], BF16, tag="wv")
                wo_sb = pool.tile([128, 8, 512], BF16, tag="wo")
                nc.sync.dma_start(out=wk_sb[:], in_=wk[:])

                qt = [
                    pool.tile([128, 2048], BF16, tag=f"qt{r}", name=f"qt{r}")
                    for r in range(4)
                ]
                kt = [
                    pool.tile([128, 2048], BF16, tag=f"kt{r}", name=f"kt{r}")
                    for r in range(4)
                ]
                v_sb = pool.tile([128, 16, 8, 65], BF16, tag="vsb")
                nc.vector.memset(v_sb[:], 1.0)
                ones = pool.tile([1, 64], BF16, tag="ones")
                nc.vector.memset(ones[:], 1.0)

                # streamed input chunks; tags sized to peak liveness
                it_map = {}

                def get_in(xkey, c):
                    if (xkey, c) not in it_map:
                        xin, tag, nb = {
                            "q": (qx, "inq", 4),
                            "k": (kx, "ink", 4),
                            "v": (vx, "inv", 2),
                        }[xkey]
                        i_t = pool.tile(
                            [128, 8, 512], BF16, tag=tag, bufs=nb,
                            name=f"i{xkey}{c}",
                        )
                        nc.sync.dma_start(
                            out=i_t[:], in_=xin[:, :, 512 * c : 512 * (c + 1)]
                        )
                        it_map[(xkey, c)] = i_t
                    return it_map[(xkey, c)]

                def proj_sub(xkey, win, dst, c, r):
                    # project input seq-chunk c for head pair r
                    i_t = get_in(xkey, c)
                    p = pp.tile([128, 512], F32, tag="pp", bufs=2)
                    for kc in range(8):
                        nc.tensor.matmul(
                            p[:],
                            win[:, kc, 128 * r : 128 * (r + 1)],
                            i_t[:, kc, :],
                            start=(kc == 0),
                            stop=(kc == 7),
                        )
                    nc.vector.tensor_copy(
                        out=dst[r][:, 512 * c : 512 * (c + 1)], in_=p[:]
                    )

                def ksub(c, r):
                    proj_sub("k", wk_sb, kt, c, r)

                def qsub(c, r):
                    proj_sub("q", wq_sb, qt, c, r)

                def vsub(c, ktl):
                    # V projection for kv chunk kti=4c+ktl, all 8 heads
                    i_t = get_in("v", c)
                    kti = 4 * c + ktl
                    p = pp.tile([128, 8, 64], F32, tag="pp", bufs=2)
                    for kc in range(8):
                        nc.tensor.matmul(
                            p[:, :, :],
                            i_t[:, kc, 128 * ktl : 128 * (ktl + 1)],
                            wv_sb[:, kc, :],
                            start=(kc == 0),
                            stop=(kc == 7),
                        )
                    nc.vector.tensor_copy(out=v_sb[:, kti, :, 0:64], in_=p[:])

                # minimal prefix: just what (qb0, r0, kti=0) needs — DMA
                # order interleaves weights with the input chunk each unit
                # actually consumes, so ksub unblocks as early as possible
                get_in("k", 0)
                nc.sync.dma_start(out=wq_sb[:], in_=wq[:])
                get_in("q", 0)
                nc.sync.dma_start(out=wv_sb[:], in_=wv[:])
                get_in("v", 0)
                ksub(0, 0)
                qsub(0, 0)
                vsub(0, 0)
                get_in("k", 1)  # pre-warm next K chunk DMA

                # Group order is r-outer ("quartets"), qb-inner: quartet 0
                # carries the K/V streaming; Q chunks and later quartets' K
                # projections spread thin across the ACT-bound groups.
                # Inserts are emitted after attnV(kti) and must precede their
                # first reader: scores(r,qb,4c) needs kt[r] chunk c -> emit at
                # kti<=4c-1; attnV(kti) needs v chunk kti -> kti-1.
                inserts = {}

                def add_ins(r, qb, kti, fn):
                    inserts.setdefault((r, qb, kti), []).append(fn)

                # v chunks stream through group (0,0), one kti ahead; DMA
                # pre-warm 4 ahead
                for kti in range(15):
                    c, ktl = divmod(kti + 1, 4)
                    add_ins(0, 0, kti, lambda c=c, k=ktl: vsub(c, k))
                for c in (1, 2, 3):
                    add_ins(0, 0, 4 * (c - 1), lambda c=c: get_in("v", c))
                # kt chunks 1..3 for r0 ahead of the sweep (DMA pre-warmed)
                for c in (1, 2, 3):
                    add_ins(0, 0, 4 * c - 2, lambda c=c: ksub(c, 0))
                for c in (2, 3):
                    add_ins(0, 0, 4 * c - 6, lambda c=c: get_in("k", c))
                # qt chunk c for r=0 lands during group (0, c-1)
                for c in (1, 2, 3):
                    add_ins(0, c - 1, 4, lambda c=c: get_in("q", c))
                    add_ins(0, c - 1, 12, lambda c=c: qsub(c, 0))
                # wo (outproj weights, first needed in quartet 3) loads after
                # the head's streaming window
                add_ins(
                    1, 0, 0,
                    lambda: nc.sync.dma_start(out=wo_sb[:], in_=wo[:]),
                )
                # quartets r>=1: kt/qt chunk 0 just before, rest spread
                for r in (1, 2, 3):
                    add_ins(r - 1, 3, 13, lambda r=r: ksub(0, r))
                    add_ins(r - 1, 3, 15, lambda r=r: qsub(0, r))
                    for c in (1, 2, 3):
                        add_ins(r, 0, 4 * c - 2, lambda c=c, r=r: ksub(c, r))
                        add_ins(r, c - 1, 8, lambda c=c, r=r: qsub(c, r))

                def outproj_dmc(prev_ot, qb, dmc):
                    p = pp.tile([128, 512], F32, tag="pp", bufs=2)
                    for r in range(4):
                        nc.tensor.matmul(
                            p[:],
                            wo_sb[
                                :,
                                2 * r + dmc // 4,
                                (dmc % 4) * 128 : (dmc % 4) * 128 + 128,
                            ],
                            prev_ot[r][:],
                            start=(r == 0),
                            stop=(r == 3),
                        )
                    yb = pool.tile([128, 512], BF16, tag="yb", bufs=2)
                    nc.vector.tensor_copy(out=yb[:], in_=p[:])
                    nc.sync.dma_start(
                        out=yT[:, dmc, 512 * qb : 512 * (qb + 1)], in_=yb[:]
                    )

                def norm_finish(acc_s, rec, ot_t):
                    # PE broadcast of 1/denom via ones outer product, then
                    # scale acc into ot.  Deferred into the next group's
                    # stream so the PE never waits on the DVE recip chain.
                    bc_ps = pp.tile([128, 512], F32, tag="pp", bufs=2)
                    nc.tensor.matmul(
                        bc_ps[0:64, :], ones[:], rec[0:1, 0:512],
                        start=True, stop=True,
                    )
                    nc.tensor.matmul(
                        bc_ps[64:128, :], ones[:], rec[0:1, 512:1024],
                        start=True, stop=True,
                    )
                    bcA = pool.tile([128, 512], F32, tag="bca", bufs=2)
                    nc.vector.tensor_copy(out=bcA[0:64, :], in_=bc_ps[0:64, :])
                    bcB = pool.tile([128, 512], F32, tag="bcb", bufs=2)
                    nc.vector.tensor_copy(
                        out=bcB[0:64, :], in_=bc_ps[64:128, :]
                    )
                    nc.vector.tensor_mul(
                        out=ot_t[0:64, :],
                        in0=acc_s[0:64, 0:512],
                        in1=bcA[0:64, :],
                    )
                    nc.vector.tensor_mul(
                        out=ot_t[64:128, :],
                        in0=acc_s[0:64, 512:1024],
                        in1=bcB[0:64, :],
                    )

                ot_store = {qb: [None] * 4 for qb in range(4)}
                pending_norm = None
                for r in range(4):
                    for qb in range(4):
                        acc = pp.tile([128, 1024], F32, tag="acc", bufs=1)
                        for kti in range(16):
                            s_t = pp.tile([128, 1024], F32, tag="sc", bufs=2)
                            nc.tensor.matmul(
                                s_t[:, 0:512],
                                kt[r][0:64, 128 * kti : 128 * (kti + 1)],
                                qt[r][0:64, 512 * qb : 512 * (qb + 1)],
                                start=True,
                                stop=True,
                            )
                            nc.tensor.matmul(
                                s_t[:, 512:1024],
                                kt[r][64:128, 128 * kti : 128 * (kti + 1)],
                                qt[r][64:128, 512 * qb : 512 * (qb + 1)],
                                start=True,
                                stop=True,
                            )
                            pt_t = pool.tile(
                                [128, 1024], BF16, tag="pt", bufs=3
                            )
                            nc.scalar.activation(
                                out=pt_t[:], in_=s_t[:], func=EXP, scale=0.125
                            )
                            for h in range(2):
                                nc.tensor.matmul(
                                    acc[0:65, 512 * h : 512 * (h + 1)],
                                    v_sb[:, kti, 2 * r + h, :],
                                    pt_t[:, 512 * h : 512 * (h + 1)],
                                    start=(kti == 0),
                                    stop=(kti == 15),
                                )
                            for fn in inserts.get((r, qb, kti), ()):
                                fn()
                            if pending_norm is not None and kti == 2:
                                pending_norm()
                                pending_norm = None
                            # finished q-chunks' output projections, spread
                            # through quartet 3's groups
                            if r == 3 and qb >= 1 and kti >= 5 and kti % 2 == 1:
                                outproj_dmc(
                                    ot_store[qb - 1], qb - 1, (kti - 5) // 2
                                )
                            if r == 3 and qb >= 2 and kti in (1, 3):
                                outproj_dmc(
                                    ot_store[qb - 2], qb - 2, 6 + (kti - 1) // 2
                                )
                        # fast eviction frees acc; normalize finish deferred
                        # into the next group's stream
                        rec = pool.tile([1, 1024], BF16, tag="rec", bufs=2)
                        with nc.allow_low_precision(
                            reason="softmax denom recip bf16"
                        ):
                            nc.vector.reciprocal(
                                out=rec[:], in_=acc[64:65, :]
                            )
                        acc_s = pool.tile(
                            [128, 1024], F32, tag="accs", bufs=2
                        )
                        nc.vector.tensor_copy(
                            out=acc_s[0:64, :], in_=acc[0:64, :]
                        )
                        ot_t = pool.tile([128, 512], BF16, tag="ot", bufs=16)
                        pending_norm = (
                            lambda a=acc_s, rc=rec, o=ot_t: norm_finish(
                                a, rc, o
                            )
                        )
                        ot_store[qb][r] = ot_t
                # tail: last normalize, then remaining output projections
                pending_norm()
                pending_norm = None
                for dmc in (6, 7):
                    outproj_dmc(ot_store[2], 2, dmc)
                for dmc in range(8):
                    outproj_dmc(ot_store[3], 3, dmc)
                it_map.clear()
    return nc


def _pack_in(x):  # [2048, 1024] -> [128, 8, 2048]
    return np.ascontiguousarray(x.T.reshape(8, 128, 2048).transpose(1, 0, 2))


def _pack_w(wt, g):  # W.T [1024,1024] cols for group g -> [128, 8, 512]
    return np.ascontiguousarray(
        wt[:, 512 * g : 512 * (g + 1)].reshape(8, 128, 512).transpose(1, 0, 2)
    )


def _pack_wo(wot, g):  # Wo.T rows for group g -> [128, 8, 512]
    a = wot[512 * g : 512 * (g + 1), :].reshape(4, 128, 1024).transpose(1, 0, 2)
    w8 = np.empty((128, 8, 512), np.float32)
    for r in range(4):
        for j in range(2):
            w8[:, 2 * r + j, :] = a[:, r, j * 512 : (j + 1) * 512]
    return w8


def _prepare(inputs, reps=1):
    query = np.asarray(inputs["query"], np.float32)
    key = np.asarray(inputs["key"], np.float32)
    value = np.asarray(inputs["value"], np.float32)
    WqT = np.asarray(inputs["Wq"], np.float32).T
    WkT = np.asarray(inputs["Wk"], np.float32).T
    WvT = np.asarray(inputs["Wv"], np.float32).T
    WoT = np.asarray(inputs["Wo"], np.float32).T

    bf = lambda a: a.astype(ml_dtypes.bfloat16)
    in_maps = []
    for c in range(8):
        b, g = c // 2, c % 2
        in_maps.append(
            {
                "qx": bf(_pack_in(query[b])),
                "kx": bf(_pack_in(key[b])),
                "vx": bf(_pack_in(value[b])),
                "wq": bf(_pack_w(WqT, g)),
                "wk": bf(_pack_w(WkT, g)),
                "wv": bf(_pack_w(WvT, g)),
                "wo": bf(_pack_wo(WoT, g)),
            }
        )

    nc = build(reps)
    nc.finalize()
    return nc, in_maps


def kernel(**inputs):
    nc, in_maps = _prepare(inputs)
    res = run_bass_kernel_spmd(nc, in_maps, core_ids=list(range(8)))

    out = np.empty((B, S, D), np.float32)
    for b in range(B):
        t = res.results[2 * b]["yT"].astype(np.float32) + res.results[
            2 * b + 1
        ]["yT"].astype(np.float32)
        out[b] = t.transpose(1, 0, 2).reshape(1024, 2048).T
    return out
